# revision 2
# baseline (speedup 1.0000x reference)
"""Trainium2 Bass kernel: 2-layer BiLSTM classifier (B=32, I=128, T=512, H=512, O=10).

Sharding: data-parallel over batch across 8 NeuronCores (b=4 per core); both
directions and both layers run locally per core; host splits/concats.

Per layer per direction:
  xp = Wih' @ input + b'                          (bulk projection -> DRAM)
  per step: g = xp_t + Whh' @ H_{t-1}             (PSUM accumulation)
  all-gate tanh trick (host pre-scales i,f,o rows by 0.5):
      t = tanh(g);  for i/f/o blocks t = 2*sigmoid(a)-1; for g block t = tanh(a)
      C_t = 0.5*(t_f+1)*C_{t-1} + (t_i+1)*t_g     [C = 2c]
      H_t = (t_o+1)*tanh(0.5*C_t)                 [H = 2h]
  The H=2h factor is absorbed into Whh/Wih1/Wlin columns (x0.5 host side).
Gate-block order is (i, f, o, g); block = PSUM bank.
"""

import numpy as np

B_FULL, I_IN, T, H, O = 32, 128, 512, 512, 10
NCORES = 8
B = B_FULL // NCORES      # 4
G4 = 4 * H                # 2048
NB = 4                    # gate banks
TBLK = 32                 # projection t-block
NTBLK = T // TBLK         # 16
PRO = 8                   # python-unrolled prologue steps
UNROLL = 8                # steps per For_i iteration

_CACHE = {}


def _build_nc():
    import concourse.bass as bass
    import concourse.mybir as mybir
    import concourse.tile as tile
    from concourse import bacc
    from concourse.bass import ds

    F32 = mybir.dt.float32
    F32R = mybir.dt.float32r
    AF = mybir.ActivationFunctionType
    OP = mybir.AluOpType

    nc = bacc.Bacc("TRN2", target_bir_lowering=False, debug=False, num_devices=NCORES)

    # ---------------- I/O ----------------
    xT_d = nc.dram_tensor("xT", [I_IN, T * B], F32R, kind="ExternalInput")
    wih0_d = {d: nc.dram_tensor(f"wih0{d}", [I_IN, G4], F32R, kind="ExternalInput") for d in "fr"}
    wih1_d = {d: nc.dram_tensor(f"wih1{d}", [128, 8 * G4], F32R, kind="ExternalInput") for d in "fr"}
    whh_d = {(l, d): nc.dram_tensor(f"whh{l}{d}", [128, 4 * G4], F32R, kind="ExternalInput")
             for l in range(2) for d in "fr"}
    b_d = {(l, d): nc.dram_tensor(f"b{l}{d}", [1, G4], F32R, kind="ExternalInput")
           for l in range(2) for d in "fr"}
    wlin_d = nc.dram_tensor("wlin", [128, 8 * O], F32R, kind="ExternalInput")
    blin_d = nc.dram_tensor("blin", [1, O], F32R, kind="ExternalInput")
    ones_d = nc.dram_tensor("ones", [1, 128], F32R, kind="ExternalInput")
    i4_d = nc.dram_tensor("i4", [B, B], F32R, kind="ExternalInput")
    out_d = nc.dram_tensor("out", [O, B], F32, kind="ExternalOutput")

    # DRAM scratch
    xp_dram = {(l, d): nc.dram_tensor(f"xp{l}{d}", [T * B, G4], F32R)
               for l in range(2) for d in "fr"}
    hs_dram = {d: nc.dram_tensor(f"hs{d}", [4, 128, T * B], F32R) for d in "fr"}

    with tile.TileContext(nc) as tc:
        import contextlib

        ctx = contextlib.ExitStack()
        sbuf = ctx.enter_context(tc.tile_pool(name="sbuf", bufs=1))
        psum = ctx.enter_context(tc.tile_pool(name="psum", bufs=1, space="PSUM"))
        xpp = ctx.enter_context(tc.tile_pool(name="xpp", bufs=3))   # 8KB slots
        tsp = ctx.enter_context(tc.tile_pool(name="tsp", bufs=2))   # 8KB slots
        evp = ctx.enter_context(tc.tile_pool(name="evp", bufs=3))   # 2KB slots
        smal = ctx.enter_context(tc.tile_pool(name="smal", bufs=2))
        wpp = ctx.enter_context(tc.tile_pool(name="wpp", bufs=2))   # 16KB slots
        xtp = ctx.enter_context(tc.tile_pool(name="xtp", bufs=2))   # 512B slots

        with ctx:
            # ---------- static tiles ----------
            ones_t = sbuf.tile([1, 128], F32R)
            nc.sync.dma_start(out=ones_t, in_=ones_d.ap())
            i4_t = sbuf.tile([B, B], F32R)
            nc.sync.dma_start(out=i4_t, in_=i4_d.ap())
            blin_t = sbuf.tile([1, O], F32R)
            nc.sync.dma_start(out=blin_t, in_=blin_d.ap())
            wlin_t = sbuf.tile([128, 8 * O], F32R)
            nc.sync.dma_start(out=wlin_t, in_=wlin_d.ap())

            hring = {d: sbuf.tile([128, 8 * 4 * B], F32R, name=f"hring_{d}") for d in "fr"}
            whh_t = {d: sbuf.tile([128, 4 * G4], F32R, name=f"whh_{d}") for d in "fr"}
            c_t = {d: sbuf.tile([B, H], F32, name=f"c_{d}") for d in "fr"}
            pooled = {d: sbuf.tile([128, 4 * B], F32, name=f"pooled_{d}") for d in "fr"}

            g_p = {}

            # ================= projection =================
            def projection(layer):
                brow = {}
                for d in "fr":
                    brow[d] = xpp.tile([1, G4], F32R, tag=f"xpc{d}", name=f"brow{d}")
                    nc.sync.dma_start(out=brow[d], in_=b_d[(layer, d)].ap())
                if layer == 0:
                    wih0_t = wpp.tile([I_IN, 2 * G4], F32R, tag="wp")
                    for di, d in enumerate("fr"):
                        nc.sync.dma_start(
                            out=wih0_t[:, G4 * di : G4 * (di + 1)], in_=wih0_d[d].ap())
                for di, d in enumerate("fr"):
                    for blk in range(NB):
                        if layer == 1:
                            wt = wpp.tile([128, 8 * 512], F32R, tag="wp")
                            for k in range(8):
                                nc.sync.dma_start(
                                    out=wt[:, 512 * k : 512 * (k + 1)],
                                    in_=wih1_d[d].ap()[:, G4 * k + 512 * blk : G4 * k + 512 * blk + 512],
                                )
                        for t0 in range(NTBLK):
                            pp = psum.tile([128, 512], F32, tag=("gf" if t0 % 2 == 0 else "gr"), name="pp")
                            nc.tensor.matmul(
                                pp, lhsT=ones_t,
                                rhs=brow[d][:, 512 * blk : 512 * (blk + 1)],
                                start=True, stop=False)
                            if layer == 0:
                                xt = xtp.tile([I_IN, TBLK * B], F32R, tag="xt")
                                nc.sync.dma_start(
                                    out=xt,
                                    in_=xT_d.ap()[:, TBLK * B * t0 : TBLK * B * (t0 + 1)])
                                nc.tensor.matmul(
                                    pp, lhsT=xt,
                                    rhs=wih0_t[:, G4 * di + 512 * blk : G4 * di + 512 * blk + 512],
                                    start=False, stop=True)
                            else:
                                for k in range(8):
                                    dd = "f" if k < 4 else "r"
                                    ht = xtp.tile([128, TBLK * B], F32R, tag="xt")
                                    nc.sync.dma_start(
                                        out=ht,
                                        in_=hs_dram[dd].ap()[k % 4, :, TBLK * B * t0 : TBLK * B * (t0 + 1)])
                                    nc.tensor.matmul(
                                        pp, lhsT=ht,
                                        rhs=wt[:, 512 * k : 512 * (k + 1)],
                                        start=False, stop=(k == 7))
                            ev = evp.tile([128, 512], F32R, tag="ev")
                            nc.scalar.activation(ev, pp, AF.Identity)
                            nc.sync.dma_start(
                                out=xp_dram[(layer, d)].ap()[
                                    TBLK * B * t0 : TBLK * B * (t0 + 1),
                                    512 * blk : 512 * (blk + 1)],
                                in_=ev)

            # ================= recurrence =================
            # Two-direction software pipeline: per step the PE FIFO sees
            # [fwd MM-group][bwd tail(t-1) transposes][bwd MM-group][fwd tail]
            # so PE never stalls on a gate chain.
            tsb_cur = {}

            def emit_mms(layer, d, is_t0, toff, pslot):
                row = (lambda a: a[toff : toff + B, :]) if isinstance(toff, int) \
                    else (lambda a: a[ds(toff, B), :])
                xpc = xpp.tile([B, G4], F32R, tag=f"xpc{d}", name=f"xpc{d}")
                nc.sync.dma_start(out=xpc, in_=row(xp_dram[(layer, d)].ap()))
                tsb = tsp.tile([B, G4], F32, tag=f"tsb{d}", name=f"tsb{d}")
                tsb_cur[d] = tsb
                for blk in range(NB):
                    if not is_t0:
                        for k in range(4):
                            nc.tensor.matmul(
                                g_p[d][0:B, blk, :],
                                lhsT=hring[d][:, pslot * 16 + 4 * k : pslot * 16 + 4 * k + 4],
                                rhs=whh_t[d][:, G4 * k + 512 * blk : G4 * k + 512 * blk + 512],
                                start=(k == 0), stop=False)
                    nc.tensor.matmul(
                        g_p[d][0:B, blk, :],
                        lhsT=i4_t,
                        rhs=xpc[:, 512 * blk : 512 * (blk + 1)],
                        start=is_t0, stop=True)
                nc.scalar.activation(
                    tsb.rearrange("b (n g) -> b n g", n=NB),
                    g_p[d][0:B, :, :], AF.Tanh)

            def emit_tail(layer, d, toff, slot):
                tsb = tsb_cur[d]
                a_t = smal.tile([B, H], F32, tag="a", name="a_t")
                nc.vector.scalar_tensor_tensor(
                    out=a_t, in0=tsb[:, 0:512], scalar=1.0,
                    in1=tsb[:, 1536:2048], op0=OP.add, op1=OP.mult)
                bb_t = smal.tile([B, H], F32, tag="bb", name="bb_t")
                nc.vector.scalar_tensor_tensor(
                    out=bb_t, in0=tsb[:, 512:1024], scalar=1.0,
                    in1=c_t[d], op0=OP.add, op1=OP.mult)
                nc.vector.scalar_tensor_tensor(
                    out=c_t[d], in0=bb_t, scalar=0.5, in1=a_t,
                    op0=OP.mult, op1=OP.add)
                tch = smal.tile([B, H], F32, tag="tc", name="tch")
                nc.scalar.activation(tch, c_t[d], AF.Tanh, scale=0.5)
                h_t = smal.tile([B, H], F32R, tag="h", name="h_t")
                nc.vector.scalar_tensor_tensor(
                    out=h_t, in0=tsb[:, 1024:1536], scalar=1.0,
                    in1=tch, op0=OP.add, op1=OP.mult)
                trv = g_p[d].bitcast(F32R)
                for k in range(4):
                    nc.tensor.transpose(
                        trv[:, 0, B * k : B * (k + 1)],
                        h_t[:, 128 * k : 128 * (k + 1)],
                        i4_t)
                nc.vector.tensor_copy(
                    hring[d][:, slot * 16 : (slot + 1) * 16], trv[:, 0, 0:16])
                if layer == 0:
                    pass
                else:
                    nc.vector.tensor_tensor(
                        out=pooled[d], in0=pooled[d], in1=g_p[d].bitcast(F32)[:, 0, 0:16],
                        op=OP.add)

            def toff_of(d, i, u):
                if i is None:
                    t = u
                    tt = t if d == "f" else T - 1 - t
                    return B * tt
                if d == "f":
                    return i * (UNROLL * B) + u * B
                return i * (-UNROLL * B) + (T - 1 - u) * B

            def recurrence(layer):
                for d in "fr":
                    g_p[d] = psum.tile([128, NB, 512], F32, tag=f"g{d}", name=f"gps_{d}")
                    nc.vector.memset(c_t[d], 0.0)
                    if layer == 1:
                        nc.vector.memset(pooled[d], 0.0)

                def steps(i, urange):
                    prev_u = None
                    for u in urange:
                        is0 = (i is None and u == 0)
                        emit_mms(layer, "f", is0, toff_of("f", i, u), (u - 1) % 8)
                        if prev_u is not None:
                            emit_tail(layer, "r", toff_of("r", i, prev_u), prev_u % 8)
                        emit_mms(layer, "r", is0, toff_of("r", i, u), (u - 1) % 8)
                        emit_tail(layer, "f", toff_of("f", i, u), u % 8)
                        prev_u = u
                    emit_tail(layer, "r", toff_of("r", i, prev_u), prev_u % 8)
                    if layer == 0:
                        # flush the 8 ring slots to DRAM (one DMA per k-chunk per dir)
                        for d in "fr":
                            dstall = hs_dram[d].ap().rearrange("k p t -> p k t")
                            for k in range(4):
                                if d == "f":
                                    base = toff_of(d, i, 0)
                                    src = bass.AP(
                                        tensor=hring[d].tensor,
                                        offset=hring[d][:, 4 * k : 4 * k + 4].offset,
                                        ap=[list(hring[d].ap[0]), [16, 8], [1, 4]],
                                    )
                                else:
                                    base = toff_of(d, i, 7)
                                    src = bass.AP(
                                        tensor=hring[d].tensor,
                                        offset=hring[d][:, 7 * 16 + 4 * k : 7 * 16 + 4 * k + 4].offset,
                                        ap=[list(hring[d].ap[0]), [-16, 8], [1, 4]],
                                    )
                                dst = dstall[:, k, base : base + 8 * B] if isinstance(base, int) \
                                    else dstall[:, k, ds(base, 8 * B)]
                                nc.scalar.dma_start(out=dst, in_=src)

                steps(None, range(PRO))
                with tc.For_i(1, T // UNROLL) as i:
                    steps(i, range(UNROLL))

            # ================= run =================
            for d in "fr":
                nc.sync.dma_start(out=whh_t[d], in_=whh_d[(0, d)].ap())
            projection(0)
            recurrence(0)
            projection(1)
            for d in "fr":
                nc.sync.dma_start(out=whh_t[d], in_=whh_d[(1, d)].ap())
            recurrence(1)

            # ---------- final linear ----------
            plr = {d: sbuf.tile([128, 4 * B], F32R, name=f"plr_{d}") for d in "fr"}
            for d in "fr":
                nc.vector.tensor_copy(plr[d], pooled[d])
            fin_ps = psum.tile([O, B], F32, tag="gf", name="fin_ps")
            nc.tensor.matmul(fin_ps, lhsT=blin_t, rhs=ones_t[:, 0:B],
                             start=True, stop=False)
            for k in range(8):
                dd = "f" if k < 4 else "r"
                nc.tensor.matmul(
                    fin_ps,
                    lhsT=wlin_t[:, O * k : O * (k + 1)],
                    rhs=plr[dd][:, B * (k % 4) : B * (k % 4 + 1)],
                    start=False, stop=(k == 7))
            fin_sb = sbuf.tile([O, B], F32)
            nc.scalar.copy(fin_sb, fin_ps)
            nc.sync.dma_start(out=out_d.ap(), in_=fin_sb)

    nc.compile()
    return nc


# ======================= host side =======================

def _prep_weights(inputs):
    f32 = np.float32
    perm = np.concatenate([np.arange(0, 1024), np.arange(1536, 2048),
                           np.arange(1024, 1536)])
    rs = np.ones(G4, f32)
    rs[:1536] = 0.5

    def whh_dev(W):
        Wp = (W[perm] * rs[:, None] * 0.5).astype(f32)     # [2048, 512]
        return np.ascontiguousarray(
            Wp.T.reshape(4, 128, G4).transpose(1, 0, 2).reshape(128, 4 * G4))

    def wih1_dev(W):
        Wp = (W[perm] * rs[:, None] * 0.5).astype(f32)     # [2048, 1024]
        return np.ascontiguousarray(
            Wp.T.reshape(8, 128, G4).transpose(1, 0, 2).reshape(128, 8 * G4))

    out = {}
    for d in "fr":
        out[f"wih0{d}"] = np.ascontiguousarray(
            (inputs[f"Wih0{d}"][perm] * rs[:, None]).astype(f32).T)
        out[f"whh0{d}"] = whh_dev(inputs[f"Whh0{d}"])
        out[f"b0{d}"] = (inputs[f"b0{d}"][perm] * rs).astype(f32)[None, :]
        out[f"wih1{d}"] = wih1_dev(inputs[f"Wih1{d}"])
        out[f"whh1{d}"] = whh_dev(inputs[f"Whh1{d}"])
        out[f"b1{d}"] = (inputs[f"b1{d}"][perm] * rs).astype(f32)[None, :]
    wl = (inputs["Wlin"] * (0.5 / T)).astype(f32)           # [10, 1024]
    out["wlin"] = np.ascontiguousarray(
        wl.T.reshape(8, 128, O).transpose(1, 0, 2).reshape(128, 8 * O))
    out["blin"] = inputs["blin"].astype(f32)[None, :]
    out["ones"] = np.ones((1, 128), f32)
    out["i4"] = np.eye(B, dtype=f32)
    return out


def _make_in_maps(inputs):
    shared = _prep_weights(inputs)
    x = np.asarray(inputs["x"], dtype=np.float32)           # [32, 128, 512]
    in_maps = []
    for c in range(NCORES):
        xs = x[B * c : B * (c + 1)]                         # [4, 128, 512]
        m = dict(shared)
        m["xT"] = np.ascontiguousarray(
            xs.transpose(1, 2, 0).reshape(I_IN, T * B))
        in_maps.append(m)
    return in_maps


def _assemble_out(results):
    out = np.zeros((B_FULL, O), np.float32)
    for c in range(NCORES):
        out[B * c : B * (c + 1)] = results[c]["out"].T
    return out


def kernel(**inputs):
    from concourse.bass_utils import run_bass_kernel_spmd

    if "nc" not in _CACHE:
        _CACHE["nc"] = _build_nc()
    nc = _CACHE["nc"]

    in_maps = _make_in_maps(inputs)
    res = run_bass_kernel_spmd(nc, in_maps, core_ids=list(range(NCORES)))
    return _assemble_out(res.results)



# revision 18
# speedup vs baseline: 1.2048x; 1.2048x over previous
"""Trainium2 Bass kernel: 2-layer BiLSTM classifier (B=32, I=128, T=512, H=512, O=10).

Sharding: 8 cores = 4 batch groups x 2 directions. Each core runs both layers
for ONE direction on 8 batch rows; bwd cores receive time-reversed input so
the on-device program is direction-agnostic. Between layers, core pairs
AllGather their layer-0 hidden states (h1^T, chunk-major) for the layer-1
input projection; the final linear is summed across the pair via AllReduce.

Per layer on each core:
  xp = Wih^T @ input + b   (bulk projection -> DRAM, streamed back per 8 steps)
  per step t: g[b,4H] = xp_t + Whh^T h_{t-1}  (PSUM, 4 gate banks i,f,g,o)
    i,f,o = sigmoid(bank), g~ = tanh(bank)    (per-bank acts, overlap MMs)
    c = f*c + i*g~; h = o*tanh(c)             (DVE + Pool)
    h^T via PE transpose -> ring -> (L0) DRAM flush per 8 steps
Gate-bank emission order g,i,f,o so the c-chain starts early.
"""

import numpy as np

B_FULL, I_IN, T, H, O = 32, 128, 512, 512, 10
NCORES = 8
B = 8                      # batch rows per core
G4 = 4 * H                 # 2048
NB = 4                     # gate banks (tile index): 0=i 1=f 2=g 3=o
UNROLL = 8
NBLK = T // UNROLL         # 64
TB = T * B                 # 4096
MT = TB // 128             # 32 m-tiles for projections
BORDER = (2, 0, 1, 3)      # bank emission order: g, i, f, o
TRB = 2                    # transposes land in bank BORDER[0]'s region

_CACHE = {}


def _build_nc(sim=False):
    # sim=True: replace collectives with local DMA stand-ins so the
    # single-core TimelineSim can run the program (analysis only)
    import contextlib

    import concourse.bass as bass
    import concourse.mybir as mybir
    import concourse.tile as tile
    from concourse import bacc
    from concourse.bass import ds

    F32 = mybir.dt.float32
    F32R = mybir.dt.float32r
    AF = mybir.ActivationFunctionType
    OP = mybir.AluOpType

    nc = bacc.Bacc("TRN2", target_bir_lowering=False, debug=False, num_devices=NCORES)

    # ---------------- I/O ----------------
    xT_d = nc.dram_tensor("xT", [I_IN, TB], F32R, kind="ExternalInput")
    wih0_d = nc.dram_tensor("wih0", [I_IN, G4], F32R, kind="ExternalInput")
    whh0_d = nc.dram_tensor("whh0", [128, 4 * G4], F32R, kind="ExternalInput")
    whh1_d = nc.dram_tensor("whh1", [128, 4 * G4], F32R, kind="ExternalInput")
    wih1o_d = nc.dram_tensor("wih1o", [128, 4 * G4], F32R, kind="ExternalInput")
    wih1x_d = nc.dram_tensor("wih1x", [128, 4 * G4], F32R, kind="ExternalInput")
    b0_d = nc.dram_tensor("b0", [1, G4], F32R, kind="ExternalInput")
    b1_d = nc.dram_tensor("b1", [1, G4], F32R, kind="ExternalInput")
    wlin_d = nc.dram_tensor("wlin", [128, 4 * O], F32R, kind="ExternalInput")
    blin_d = nc.dram_tensor("blin", [1, O], F32R, kind="ExternalInput")
    ones_d = nc.dram_tensor("ones", [1, 128], F32R, kind="ExternalInput")
    i64_d = nc.dram_tensor("i64", [64, 64], F32R, kind="ExternalInput")
    i8_d = nc.dram_tensor("i8", [B, B], F32R, kind="ExternalInput")
    out_d = nc.dram_tensor("out", [O, B], F32, kind="ExternalOutput")

    # ---------------- DRAM scratch ----------------
    xp_dram = nc.dram_tensor("xp", [TB, G4], F32R)         # shared by both layers
    hs_dram = nc.dram_tensor("hs", [4, 128, TB], F32R)     # own h1^T; AG input
    ag_out = nc.dram_tensor("agout", [2 * 4 * 128, TB], F32R)
    fin_in = nc.dram_tensor("fin_in", [O, B], F32)
    fin_out = nc.dram_tensor("fin_out", [O, B], F32)
    groups = [[2 * p, 2 * p + 1] for p in range(4)]

    with tile.TileContext(nc) as tc:
        ctx = contextlib.ExitStack()
        sbuf = ctx.enter_context(tc.tile_pool(name="sbuf", bufs=1))
        psum = ctx.enter_context(tc.tile_pool(name="psum", bufs=1, space="PSUM"))
        xpp = ctx.enter_context(tc.tile_pool(name="xpp", bufs=3))
        gat = ctx.enter_context(tc.tile_pool(name="gat", bufs=2))
        smal = ctx.enter_context(tc.tile_pool(name="smal", bufs=2))
        prjp = ctx.enter_context(tc.tile_pool(name="prj", bufs=2))

        with ctx:
            # ---------- static tiles ----------
            ones_t = sbuf.tile([1, 128], F32R)
            nc.sync.dma_start(out=ones_t, in_=ones_d.ap())
            i64_t = sbuf.tile([64, 64], F32R)
            nc.sync.dma_start(out=i64_t, in_=i64_d.ap())
            i8_t = sbuf.tile([B, B], F32R)
            nc.sync.dma_start(out=i8_t, in_=i8_d.ap())
            blin_t = sbuf.tile([1, O], F32R)
            nc.sync.dma_start(out=blin_t, in_=blin_d.ap())
            wlin_t = sbuf.tile([128, 4 * O], F32R)
            nc.sync.dma_start(out=wlin_t, in_=wlin_d.ap())
            xT_t = sbuf.tile([I_IN, TB], F32R)
            nc.sync.dma_start(out=xT_t, in_=xT_d.ap())
            wih0_t = sbuf.tile([I_IN, G4], F32R)
            nc.sync.dma_start(out=wih0_t, in_=wih0_d.ap())
            b_t = {}
            for lb, src in ((0, b0_d), (1, b1_d)):
                b_t[lb] = sbuf.tile([1, G4], F32R, name=f"b{lb}")
                nc.sync.dma_start(out=b_t[lb], in_=src.ap())
            # one big weight buffer, reloaded per phase:
            # whh0 -> wih1 own -> wih1 other -> whh1
            wbig = sbuf.tile([128, 4 * G4], F32R, name="wbig")
            nc.gpsimd.dma_start(out=wbig, in_=whh0_d.ap())

            ring = sbuf.tile([128, UNROLL * 4 * B], F32R, name="ring")
            h_st = [sbuf.tile([B, H], F32R, name="h_even"),
                    sbuf.tile([B, H], F32R, name="h_odd")]
            c_t = sbuf.tile([B, H], F32, name="c")
            pooled = sbuf.tile([128, 4 * B], F32, name="pooled")

            st = {}  # per-step state passed between emission segments
            gps = {}  # persistent psum group tiles, keyed by step parity

            # ================= recurrence =================
            def emit_mms(layer, u, first, xp_blk):
                Gt = gps[u % 2]
                prev = (u - 1) % UNROLL
                for bank in BORDER:
                    nc.tensor.matmul(
                        Gt[bank][0:B, :],
                        lhsT=i64_t[:, 8 * u : 8 * u + 8],
                        rhs=xp_blk[:, 512 * bank : 512 * (bank + 1)],
                        start=True, stop=first)
                # previous step's transposes + ring copy go between the xp
                # MMs (independent) and the whh MMs (which need the ring)
                emit_prev_tail(layer)
                acts = {}
                tmp = smal.tile([B, H], F32, tag="tmp", name="tmp")
                for bank in BORDER[:3]:
                    if not first:
                        for k in range(4):
                            nc.tensor.matmul(
                                Gt[bank][0:B, :],
                                lhsT=ring[:, prev * 32 + 8 * k : prev * 32 + 8 * k + 8],
                                rhs=wbig[:, k * G4 + 512 * bank : k * G4 + 512 * bank + 512],
                                start=False, stop=(k == 3))
                    t_ = gat.tile([B, 512], F32, tag=f"t{bank}", name=f"t{bank}")
                    nc.scalar.activation(
                        t_, Gt[bank][0:B, :], AF.Tanh if bank == 2 else AF.Sigmoid)
                    acts[bank] = t_
                    if bank == 0:
                        # i*g~ on Pool in halves, early and off the chain
                        for hh in (0, 1):
                            sl = slice(256 * hh, 256 * (hh + 1))
                            nc.gpsimd.tensor_tensor(
                                out=tmp[:, sl], in0=acts[0][:, sl],
                                in1=acts[2][:, sl], op=OP.mult)
                if not first:
                    for k in range(4):
                        nc.tensor.matmul(
                            Gt[3][0:B, :],
                            lhsT=ring[:, prev * 32 + 8 * k : prev * 32 + 8 * k + 8],
                            rhs=wbig[:, k * G4 + 512 * 3 : k * G4 + 512 * 3 + 512],
                            start=False, stop=(k == 3))
                # c/h chain in halves: DVE cf/add, act o/tanh interleaved
                to = gat.tile([B, 512], F32, tag="t3", name="t3")
                cf = smal.tile([B, H], F32, tag="cf", name="cf")
                tch = smal.tile([B, H], F32, tag="tch", name="tch")
                h_t = h_st[u % 2]
                for hh in (0, 1):
                    sl = slice(256 * hh, 256 * (hh + 1))
                    nc.vector.tensor_tensor(
                        out=cf[:, sl], in0=acts[1][:, sl], in1=c_t[:, sl], op=OP.mult)
                    nc.vector.tensor_tensor(
                        out=c_t[:, sl], in0=cf[:, sl], in1=tmp[:, sl], op=OP.add)
                    nc.scalar.activation(to[:, sl], Gt[3][0:B, sl], AF.Sigmoid)
                    nc.scalar.activation(tch[:, sl], c_t[:, sl], AF.Tanh)
                for hh in (0, 1):
                    sl = slice(256 * hh, 256 * (hh + 1))
                    nc.vector.tensor_tensor(
                        out=h_t[:, sl], in0=to[:, sl], in1=tch[:, sl], op=OP.mult)
                st["h"] = h_t
                st["G"] = Gt
                st["u"] = u

            def emit_prev_tail(layer):
                if "h" not in st:
                    return
                h_t, Gt, u = st.pop("h"), st.pop("G"), st.pop("u")
                trv = Gt[TRB].bitcast(F32R)
                slot = u % UNROLL
                for k in range(4):
                    nc.tensor.transpose(
                        trv[:, 8 * k : 8 * k + 8],
                        h_t[:, 128 * k : 128 * (k + 1)], i8_t)
                    if k % 2 == 1:
                        nc.vector.tensor_copy(
                            ring[:, slot * 32 + 8 * (k - 1) : slot * 32 + 8 * (k + 1)],
                            trv[:, 8 * (k - 1) : 8 * (k + 1)])
                if layer == 1:
                    nc.vector.tensor_tensor(
                        out=pooled, in0=pooled, in1=Gt[TRB][:, 0:32], op=OP.add)

            def emit_flush(i):
                # block i's ring slots -> hs_dram (i static or loop reg)
                dstall = hs_dram.ap().rearrange("k p t -> p k t")
                for k in range(4):
                    src = bass.AP(
                        tensor=ring.tensor,
                        offset=ring[:, 8 * k : 8 * k + 8].offset,
                        ap=[list(ring.ap[0]), [32, UNROLL], [1, 8]])
                    dst = (dstall[:, k, 64 * i : 64 * (i + 1)] if isinstance(i, int)
                           else dstall[:, k, ds(i * 64, 64)])
                    nc.scalar.dma_start(out=dst, in_=src)

            def recurrence(layer):
                nc.vector.memset(c_t, 0.0)
                if layer == 1:
                    nc.vector.memset(pooled, 0.0)
                st.clear()
                gps[0] = [psum.tile([128, 512], F32, tag=f"gA{b}", name=f"gA{b}")
                          for b in range(NB)]
                gps[1] = [psum.tile([128, 512], F32, tag=f"gB{b}", name=f"gB{b}")
                          for b in range(NB)]

                def block(i, first_block):
                    xp_blk = xpp.tile([64, G4], F32R, tag="xpb", name="xpb")
                    if first_block:
                        nc.sync.dma_start(out=xp_blk, in_=xp_dram.ap()[0:64, :])
                    else:
                        nc.sync.dma_start(
                            out=xp_blk, in_=xp_dram.ap()[ds(i * 64, 64), :])
                    emit_mms(layer, 0, first_block, xp_blk)
                    if layer == 0 and not first_block:
                        # flush the previous block now: its slot-7 ring copy
                        # was just emitted, and this block's steps are about
                        # to overwrite slots 0..6
                        emit_flush(i - 1)
                    for u in range(1, UNROLL):
                        emit_mms(layer, u, False, xp_blk)

                block(0, True)
                with tc.For_i(1, NBLK) as i:
                    block(i, False)
                emit_prev_tail(layer)          # final step's transposes
                if layer == 0:
                    emit_flush(NBLK - 1)

            # ================= projections =================
            def proj_psum():
                return [[psum.tile([128, 512], F32, tag=f"g{pc}{b}", name=f"pp{pc}{b}")
                         for b in range(NB)] for pc in "AB"]

            def proj0():
                ppg = proj_psum()
                for m in range(MT):
                    pp = ppg[m % 2]
                    ev = prjp.tile([128, G4], F32R, tag="ev", name="ev")
                    for bank in range(NB):
                        nc.tensor.matmul(
                            pp[bank], lhsT=ones_t[0:1, 0:128],
                            rhs=b_t[0][:, 512 * bank : 512 * (bank + 1)],
                            start=True, stop=False)
                        nc.tensor.matmul(
                            pp[bank],
                            lhsT=xT_t[:, 128 * m : 128 * (m + 1)],
                            rhs=wih0_t[:, 512 * bank : 512 * (bank + 1)],
                            start=False, stop=True)
                        nc.scalar.activation(
                            ev[:, 512 * bank : 512 * (bank + 1)], pp[bank], AF.Identity)
                    nc.gpsimd.dma_start(
                        out=xp_dram.ap()[128 * m : 128 * (m + 1), :], in_=ev)

            def proj1_own():
                ppg = proj_psum()
                for m in range(MT):
                    hto = prjp.tile([128, 512], F32R, tag="hto", name="hto")
                    for k in range(4):
                        nc.sync.dma_start(
                            out=hto[:, 128 * k : 128 * (k + 1)],
                            in_=hs_dram.ap()[k, :, 128 * m : 128 * (m + 1)])
                    pp = ppg[m % 2]
                    ev = prjp.tile([128, G4], F32R, tag="ev", name="ev")
                    for bank in range(NB):
                        nc.tensor.matmul(
                            pp[bank], lhsT=ones_t[0:1, 0:128],
                            rhs=b_t[1][:, 512 * bank : 512 * (bank + 1)],
                            start=True, stop=False)
                        for k in range(4):
                            nc.tensor.matmul(
                                pp[bank],
                                lhsT=hto[:, 128 * k : 128 * (k + 1)],
                                rhs=wbig[:, k * G4 + 512 * bank : k * G4 + 512 * bank + 512],
                                start=False, stop=(k == 3))
                        nc.scalar.activation(
                            ev[:, 512 * bank : 512 * (bank + 1)], pp[bank], AF.Identity)
                    nc.gpsimd.dma_start(
                        out=xp_dram.ap()[128 * m : 128 * (m + 1), :], in_=ev)

            def proj1_other():
                pid = nc.sync.partition_id()
                roff = (1 - pid % 2) * 512
                ppg = proj_psum()
                for m in range(MT):
                    hto = prjp.tile([128, 512], F32R, tag="hto", name="hto")
                    for k in range(4):
                        nc.sync.dma_start(
                            out=hto[:, 128 * k : 128 * (k + 1)],
                            in_=ag_out.ap()[ds(roff + 128 * k, 128),
                                            128 * (MT - 1 - m) : 128 * (MT - m)])
                    xpo = prjp.tile([128, G4], F32R, tag="xpo", name="xpo")
                    nc.scalar.dma_start(
                        out=xpo, in_=xp_dram.ap()[128 * m : 128 * (m + 1), :])
                    # reverse the partner tile into my time order (DVE copy,
                    # matmul operand APs must stay 2D)
                    hto2 = prjp.tile([128, 512], F32R, tag="hto2", name="hto2")
                    rev = bass.AP(
                        tensor=hto.tensor,
                        offset=hto[:, 120:128].offset,
                        ap=[list(hto.ap[0]), [128, 4], [-8, 16], [1, 8]])
                    nc.vector.tensor_copy(
                        hto2.rearrange("p (a b c) -> p a b c", a=4, b=16, c=8), rev)
                    pp = ppg[m % 2]
                    ev = prjp.tile([128, G4], F32R, tag="ev", name="ev")
                    for bank in range(NB):
                        for k in range(4):
                            nc.tensor.matmul(
                                pp[bank],
                                lhsT=hto2[:, 128 * k : 128 * (k + 1)],
                                rhs=wbig[:, k * G4 + 512 * bank : k * G4 + 512 * bank + 512],
                                start=(k == 0), stop=(k == 3))
                        nc.vector.tensor_tensor(
                            out=ev[:, 512 * bank : 512 * (bank + 1)], in0=pp[bank],
                            in1=xpo[:, 512 * bank : 512 * (bank + 1)], op=OP.add)
                    nc.gpsimd.dma_start(
                        out=xp_dram.ap()[128 * m : 128 * (m + 1), :], in_=ev)

            # ================= run =================
            proj0()
            recurrence(0)
            if sim:
                ago = ag_out.ap().rearrange("(r x) t -> r x t", r=2)
                nc.gpsimd.dma_start(out=ago[0], in_=hs_dram.ap().rearrange("k p t -> (k p) t"))
                nc.gpsimd.dma_start(out=ago[1], in_=hs_dram.ap().rearrange("k p t -> (k p) t"))
            else:
                nc.gpsimd.collective_compute(
                    "AllGather", mybir.AluOpType.bypass, replica_groups=groups,
                    ins=[hs_dram.ap()], outs=[ag_out.ap()])
            nc.gpsimd.dma_start(out=wbig, in_=wih1o_d.ap())
            proj1_own()
            nc.gpsimd.dma_start(out=wbig, in_=wih1x_d.ap())
            proj1_other()
            nc.gpsimd.dma_start(out=wbig, in_=whh1_d.ap())
            recurrence(1)

            # ---------- final linear ----------
            plr = sbuf.tile([128, 4 * B], F32R, name="plr")
            nc.vector.tensor_copy(plr, pooled)
            fin_full = psum.tile([128, 512], F32, tag="gA0", name="fin_full")
            fin_ps = fin_full[0:O, 0:B]
            nc.tensor.matmul(fin_ps, lhsT=blin_t, rhs=ones_t[0:1, 0:B],
                             start=True, stop=False)
            for k in range(4):
                nc.tensor.matmul(
                    fin_ps, lhsT=wlin_t[:, O * k : O * (k + 1)],
                    rhs=plr[:, B * k : B * (k + 1)],
                    start=False, stop=(k == 3))
            fin_sb = sbuf.tile([O, B], F32, name="fin_sb")
            nc.scalar.copy(fin_sb, fin_ps)
            nc.sync.dma_start(out=fin_in.ap(), in_=fin_sb)
            if sim:
                nc.gpsimd.dma_start(out=fin_out.ap(), in_=fin_in.ap())
            else:
                nc.gpsimd.collective_compute(
                    "AllReduce", mybir.AluOpType.add, replica_groups=groups,
                    ins=[fin_in.ap()], outs=[fin_out.ap()])
            fin2 = sbuf.tile([O, B], F32, name="fin2")
            nc.sync.dma_start(out=fin2, in_=fin_out.ap())
            nc.sync.dma_start(out=out_d.ap(), in_=fin2)

    nc.compile()
    return nc


# ======================= host side =======================

def _chunked(W):
    # W: [4H, K] -> [128, (K/128)*4H] chunk-major along K
    f32 = np.float32
    K = W.shape[1]
    return np.ascontiguousarray(
        W.astype(f32).T.reshape(K // 128, 128, G4).transpose(1, 0, 2)
        .reshape(128, (K // 128) * G4))


def _make_in_maps(inputs):
    f32 = np.float32
    x = np.asarray(inputs["x"], f32)
    maps = []
    for c in range(NCORES):
        p, r = c // 2, c % 2
        d = "f" if r == 0 else "r"
        xs = x[B * p : B * (p + 1)]                  # [8, 128, 512]
        arr = xs.transpose(1, 2, 0)                  # [128, T, 8]
        if r == 1:
            arr = arr[:, ::-1, :]
        m = {"xT": np.ascontiguousarray(arr.reshape(I_IN, TB))}
        m["wih0"] = np.ascontiguousarray(inputs[f"Wih0{d}"].astype(f32).T)
        m["whh0"] = _chunked(inputs[f"Whh0{d}"])
        m["whh1"] = _chunked(inputs[f"Whh1{d}"])
        m["b0"] = inputs[f"b0{d}"].astype(f32)[None, :]
        m["b1"] = inputs[f"b1{d}"].astype(f32)[None, :]
        W1 = inputs[f"Wih1{d}"].astype(f32)          # [2048, 1024]
        m["wih1o"] = _chunked(W1[:, 512 * r : 512 * (r + 1)])
        m["wih1x"] = _chunked(W1[:, 512 * (1 - r) : 512 * (2 - r)])
        wl = (inputs["Wlin"].astype(f32)[:, 512 * r : 512 * (r + 1)] / T)
        m["wlin"] = np.ascontiguousarray(
            wl.T.reshape(4, 128, O).transpose(1, 0, 2).reshape(128, 4 * O))
        m["blin"] = (inputs["blin"].astype(f32) * 0.5)[None, :]
        m["ones"] = np.ones((1, 128), f32)
        m["i64"] = np.eye(64, dtype=f32)
        m["i8"] = np.eye(B, dtype=f32)
        maps.append(m)
    return maps


def _assemble_out(results):
    out = np.zeros((B_FULL, O), np.float32)
    for p in range(4):
        out[B * p : B * (p + 1)] = results[2 * p]["out"].T
    return out


def kernel(**inputs):
    from concourse.bass_utils import run_bass_kernel_spmd

    if "nc" not in _CACHE:
        _CACHE["nc"] = _build_nc()
    nc = _CACHE["nc"]

    in_maps = _make_in_maps(inputs)
    res = run_bass_kernel_spmd(nc, in_maps, core_ids=list(range(NCORES)))
    return _assemble_out(res.results)
